# revision 3
# baseline (speedup 1.0000x reference)
"""Causal attention kernel for TRN2, 8 NeuronCores, sequence-parallel.

Problem: x[8192,1024] @ Wqkv[1024,192] -> q,k,v[8192,64];
         causal softmax(q k^T / 8) @ v -> [8192,64]; @ Wout[64,1024] + bout.

Sharding: 16 query blocks of 512 rows; core m owns blocks m and 15-m, so
every core processes the same number of unmasked KV columns (17*512).
Each core projects q/k/v for its own rows, all-gathers kT and v, then runs
a flat 68-chunk score/softmax/PV loop (chunk = 128 KV rows x 512 queries).

Everything is computed transposed: qT/kT [64, rows] so that
  scoresT[j,q] = kT_chunk.T-matmul with qT as moving operand,
  exp runs PSUM->SBUF on ACT (scale=1/8 folded in),
  attn@V uses the exp tile directly as moving operand (no transposes),
  a ones-column appended to V yields softmax denominators for free, and
  1/denominator + bias are folded into the output-projection epilogue.

Per-core schedule differences (which chunk belongs to which query half,
causal mask slice) are data: an int32 table drives register-offset APs.
"""
import numpy as np
from contextlib import ExitStack

import concourse.bass as bass
import concourse.mybir as mybir
import concourse.tile as tile
from concourse import bacc
from concourse.bass_utils import run_bass_kernel_spmd
from concourse.masks import make_identity

F32 = mybir.dt.float32
I32 = mybir.dt.int32
PE = mybir.EngineType.PE
DVE = mybir.EngineType.DVE
POOL = mybir.EngineType.Pool

N, DIM, DH, DOUT = 8192, 1024, 64, 1024
NCORES = 8
QB = 512                 # query block rows
NBLK = N // QB           # 16
JT = 128                 # kv chunk width
NCH = 68                 # chunks per core: 4*(m+1) + 4*(16-m)
UW = 1408                # mask tensor width: diag slices at 384-128t, ones at 896
SCALE = DH ** -0.5


def _schedule(m: int) -> np.ndarray:
    """Per-core chunk table: columns (hq, kc, vc, mo)."""
    rows = []
    for h, b in ((0, m), (1, NBLK - 1 - m)):
        for jt in range(4 * (b + 1)):
            hq = h * QB
            kc = jt * JT
            vc = jt * (DH + 1)
            mo = 896 if jt < 4 * b else 384 - 128 * (jt - 4 * b)
            rows.append((hq, kc, vc, mo))
    assert len(rows) == NCH
    return np.asarray(rows, dtype=np.int32).reshape(1, NCH * 4)


def build():
    nc = bacc.Bacc("TRN2", target_bir_lowering=False, debug=False,
                   num_devices=NCORES)
    x = nc.dram_tensor("x", [2 * QB, DIM], F32, kind="ExternalInput")
    wqkv = nc.dram_tensor("wqkv", [DIM, 3 * DH], F32, kind="ExternalInput")
    wout = nc.dram_tensor("wout", [DH, DOUT], F32, kind="ExternalInput")
    bout = nc.dram_tensor("bout", [DOUT], F32, kind="ExternalInput")
    sched = nc.dram_tensor("sched", [1, NCH * 4], I32, kind="ExternalInput")
    out = nc.dram_tensor("out", [2 * QB, DOUT], F32, kind="ExternalOutput")

    with tile.TileContext(nc) as tc, ExitStack() as ctx:
        con = ctx.enter_context(tc.tile_pool(name="con", bufs=1))
        dram = ctx.enter_context(tc.tile_pool(name="dram", bufs=1, space="DRAM"))

        # ---- constants / inputs resident in SBUF ----
        ident = con.tile([128, 128], F32)
        make_identity(nc, ident[:])
        umask = con.tile([128, UW], F32)
        nc.gpsimd.memset(umask[:], 1.0)
        nc.gpsimd.affine_select(
            out=umask[:], in_=umask[:], pattern=[[1, UW]],
            channel_multiplier=-1, base=-384,
            compare_op=mybir.AluOpType.is_ge, fill=0.0)

        w_sb = con.tile([128, 8, 3 * DH], F32)
        nc.sync.dma_start(w_sb[:], wqkv.ap().rearrange("(c p) n -> p c n", p=128))
        wo_sb = con.tile([DH + 1, DOUT], F32)
        nc.sync.dma_start(wo_sb[0:DH, :], wout.ap())
        nc.sync.dma_start(wo_sb[DH:DH + 1, :], bout.ap()[None, :])
        t_sb = con.tile([1, NCH * 4], I32)
        nc.sync.dma_start(t_sb[:], sched.ap())

        qt_sb = con.tile([DH, 2 * QB], F32)
        kt_full = con.tile([DH, N], F32)
        v_sb = con.tile([128, (N // JT) * (DH + 1)], F32)
        v_view = v_sb.rearrange("p (t c) -> p t c", c=DH + 1)
        nc.vector.memset(v_view[:, :, DH:DH + 1], 1.0)

        gin_k = dram.tile([DH, 2 * QB], F32)
        gout_k = dram.tile([NCORES * DH, 2 * QB], F32, addr_space="Shared")
        gin_v = dram.tile([128, 8 * DH], F32)
        gout_v = dram.tile([NCORES * 128, 8 * DH], F32, addr_space="Shared")

        # ---- phase A: project own rows (transposed via PE) ----
        with tc.tile_pool(name="pa_sb", bufs=2) as pa, \
             tc.tile_pool(name="pa_ps", bufs=2, space="PSUM") as pps, \
             tc.tile_pool(name="pa_ps2", bufs=2, space="PSUM") as pps2:
            x_sb = pa.tile([128, 8, DIM], F32, tag="xin")
            nc.sync.dma_start(x_sb[:], x.ap().rearrange("(t p) d -> p t d", p=128))
            xt_sb = pa.tile([128, 8, 2 * QB], F32, tag="xt")
            for t in range(8):
                for dc in range(8):
                    tp = pps.tile([128, 128], F32, tag="tp")
                    nc.tensor.transpose(tp[:], x_sb[:, t, dc * 128:(dc + 1) * 128],
                                        ident[:])
                    dst = xt_sb[:, dc, t * 128:(t + 1) * 128]
                    if (t * 8 + dc) % 2 == 0:
                        nc.scalar.copy(dst, tp[:])
                    else:
                        nc.vector.tensor_copy(dst, tp[:])

            # qT / kT_own: [64, 1024] each
            for which, base in (("q", 0), ("k", DH)):
                for h in range(2):
                    pq = pps2.tile([DH, QB], F32, tag="pq")
                    for dc in range(8):
                        nc.tensor.matmul(
                            pq[:], w_sb[:, dc, base:base + DH],
                            xt_sb[:, dc, h * QB:(h + 1) * QB],
                            start=(dc == 0), stop=(dc == 7))
                    if which == "q":
                        nc.scalar.copy(qt_sb[:, h * QB:(h + 1) * QB], pq[:])
                    else:
                        ks = pa.tile([DH, QB], F32, tag="kstage")
                        nc.vector.tensor_copy(ks[:], pq[:])
                        nc.sync.dma_start(gin_k[:, h * QB:(h + 1) * QB], ks[:])
            nc.gpsimd.collective_compute(
                "AllGather", mybir.AluOpType.bypass,
                replica_groups=[list(range(NCORES))],
                ins=[gin_k[:]], outs=[gout_k[:]])

            # v_own rows: [128, 64] per row-tile
            for t in range(8):
                pv = pps2.tile([128, DH], F32, tag="pv")
                for dc in range(8):
                    nc.tensor.matmul(
                        pv[:], xt_sb[:, dc, t * 128:(t + 1) * 128],
                        w_sb[:, dc, 2 * DH:3 * DH],
                        start=(dc == 0), stop=(dc == 7))
                vs = pa.tile([128, DH], F32, tag="vstage")
                nc.scalar.copy(vs[:], pv[:])
                nc.sync.dma_start(gin_v[:, t * DH:(t + 1) * DH], vs[:])
            nc.gpsimd.collective_compute(
                "AllGather", mybir.AluOpType.bypass,
                replica_groups=[list(range(NCORES))],
                ins=[gin_v[:]], outs=[gout_v[:]])

            # unpack gathers into global layout
            for g in range(NCORES):
                b0, b1 = g, NBLK - 1 - g
                src = gout_k[g * DH:(g + 1) * DH, :]
                nc.sync.dma_start(kt_full[:, b0 * QB:(b0 + 1) * QB], src[:, 0:QB])
                nc.sync.dma_start(kt_full[:, b1 * QB:(b1 + 1) * QB], src[:, QB:2 * QB])
                vsrc = gout_v[g * 128:(g + 1) * 128, :]
                for half, b in ((0, b0), (1, b1)):
                    s3 = vsrc[:, half * 4 * DH:(half + 1) * 4 * DH]
                    nc.sync.dma_start(
                        v_view[:, 4 * b:4 * b + 4, 0:DH],
                        s3.rearrange("p (t c) -> p t c", c=DH))

        # ---- phase B: 68 chunks of scoresT -> exp -> mask -> attnV ----
        with tc.tile_pool(name="pb_st", bufs=3, space="PSUM") as pst, \
             tc.tile_pool(name="pb_oe", bufs=1, space="PSUM") as poe, \
             tc.tile_pool(name="pb_t1", bufs=1, space="PSUM") as pt1, \
             tc.tile_pool(name="pb_pp", bufs=1, space="PSUM") as ppp, \
             tc.tile_pool(name="pb_sb", bufs=4) as pb, \
             tc.tile_pool(name="pc_sb", bufs=2) as pc:
            zrow = pc.tile([1, DH + 1], F32, tag="zr")
            zcol = pc.tile([1, QB], F32, tag="zc")
            nc.vector.memset(zrow[:], 0.0)
            nc.vector.memset(zcol[:], 0.0)

            oe = poe.tile([DH + 1, 2 * QB], F32)
            nc.tensor.matmul(oe[:, 0:QB], zrow[:], zcol[:], start=True, stop=False)
            nc.tensor.matmul(oe[:, QB:2 * QB], zrow[:], zcol[:], start=True, stop=False)

            for c in range(NCH):
                hq = nc.values_load(t_sb[0:1, 4 * c:4 * c + 1], engines=[PE],
                                    min_val=0, max_val=QB,
                                    skip_runtime_bounds_check=True)
                kc = nc.values_load(t_sb[0:1, 4 * c + 1:4 * c + 2], engines=[POOL],
                                    min_val=0, max_val=N - JT,
                                    skip_runtime_bounds_check=True)
                vc = nc.values_load(t_sb[0:1, 4 * c + 2:4 * c + 3], engines=[POOL],
                                    min_val=0, max_val=(N // JT - 1) * (DH + 1),
                                    skip_runtime_bounds_check=True)
                mo = nc.values_load(t_sb[0:1, 4 * c + 3:4 * c + 4], engines=[DVE],
                                    min_val=0, max_val=UW - QB,
                                    skip_runtime_bounds_check=True)

                kstg = pb.tile([DH, JT], F32, tag="kstg")
                nc.gpsimd.tensor_copy(kstg[:], kt_full[:, bass.ds(kc, JT)])
                vstg = pb.tile([128, DH + 1], F32, tag="vstg")
                nc.gpsimd.tensor_copy(vstg[:], v_sb[:, bass.ds(vc, DH + 1)])

                st = pst.tile([128, QB], F32, tag="st")
                nc.tensor.matmul(st[:], kstg[:], qt_sb[:, bass.ds(hq, QB)],
                                 start=True, stop=True)
                et = pb.tile([128, QB], F32, tag="et")
                nc.scalar.activation(et[:], st[:],
                                     mybir.ActivationFunctionType.Exp, scale=SCALE)
                nc.vector.tensor_mul(et[:], et[:], umask[:, bass.ds(mo, QB)])
                nc.tensor.matmul(oe[:, bass.ds(hq, QB)], vstg[:], et[:],
                                 start=False, stop=False)

            nc.tensor.matmul(oe[:, 0:QB], zrow[:], zcol[:], start=False, stop=True)
            nc.tensor.matmul(oe[:, QB:2 * QB], zrow[:], zcol[:], start=False, stop=True)

            # ---- phase C: denominators -> [q-partition] reciprocal ----
            oe_sb = pc.tile([DH + 1, 2 * QB], F32, tag="oesb")
            nc.scalar.copy(oe_sb[:], oe[:])
            sums_t = pc.tile([128, 8], F32, tag="sums")
            for s in range(8):
                t1 = pt1.tile([128, 1], F32, tag="t1")
                nc.tensor.transpose(t1[:], oe_sb[DH:DH + 1, s * 128:(s + 1) * 128],
                                    ident[DH:DH + 1, DH:DH + 1])
                nc.vector.tensor_copy(sums_t[:, s:s + 1], t1[:])
            recip = pc.tile([128, 8], F32, tag="recip")
            nc.vector.reciprocal(recip[:], sums_t[:])

            # ---- phase D: output projection + scale epilogue ----
            for s in range(8):
                pp = ppp.tile([128, DOUT], F32, tag="pp")
                for nn in range(2):
                    nc.tensor.matmul(pp[:, nn * QB:(nn + 1) * QB],
                                     oe_sb[:, s * 128:(s + 1) * 128],
                                     wo_sb[:, nn * QB:(nn + 1) * QB],
                                     start=True, stop=True)
                fo = pb.tile([128, DOUT], F32, tag="fo")
                nc.vector.tensor_scalar_mul(fo[:], pp[:], recip[:, s:s + 1])
                nc.sync.dma_start(out.ap()[s * 128:(s + 1) * 128, :], fo[:])

    nc.compile()
    return nc


_NC_CACHE = None


def kernel(x, Wqkv, Wout, bout):
    global _NC_CACHE
    if _NC_CACHE is None:
        _NC_CACHE = build()
    nc = _NC_CACHE
    x = np.ascontiguousarray(x, dtype=np.float32)
    in_maps = []
    for m in range(NCORES):
        b0, b1 = m, NBLK - 1 - m
        x_own = np.concatenate(
            [x[b0 * QB:(b0 + 1) * QB], x[b1 * QB:(b1 + 1) * QB]], axis=0)
        in_maps.append({
            "x": x_own,
            "wqkv": np.ascontiguousarray(Wqkv, dtype=np.float32),
            "wout": np.ascontiguousarray(Wout, dtype=np.float32),
            "bout": np.ascontiguousarray(bout, dtype=np.float32),
            "sched": _schedule(m),
        })
    res = run_bass_kernel_spmd(nc, in_maps, core_ids=list(range(NCORES)))
    out = np.empty((N, DOUT), dtype=np.float32)
    for m in range(NCORES):
        b0, b1 = m, NBLK - 1 - m
        r = res.results[m]["out"]
        out[b0 * QB:(b0 + 1) * QB] = r[0:QB]
        out[b1 * QB:(b1 + 1) * QB] = r[QB:2 * QB]
    return out


# revision 13
# speedup vs baseline: 1.5750x; 1.5750x over previous
"""Causal attention kernel for TRN2, 8 NeuronCores, sequence-parallel.

Problem: x[8192,1024] @ Wqkv[1024,192] -> q,k,v[8192,64];
         causal softmax(q k^T / 8) @ v -> [8192,64]; @ Wout[64,1024] + bout.

Sharding: 16 query blocks of 512 rows; core m owns blocks m and 15-m, so
every core processes the same number of unmasked KV columns (17*512).
Each core projects q/k/v (transposed, via PE) for its own 1024 rows,
all-gathers kT and v, then runs a flat 68-chunk scoresT/exp/PV loop
(chunk = 128 KV rows x 512 queries), all matmuls in float32r.

The per-core causal structure is folded into DATA, keeping the SPMD
instruction stream uniform:
  - gathered kT/v are unpacked by dynamic-destination DMAs directly into
    per-core SCHEDULE order (k_sched/v_sched), so the inner loop's
    stationary operands are statically addressed;
  - chunks 0-7 are the two 512x512 diagonal blocks (statically masked,
    fed from locally computed kT/v so they can run during the gathers);
  - chunks 8-67 are fully-valid off-diagonal chunks, no masking, with
    only the query-half offset (hq) read from a small int32 table into
    registers (batched loads of 4).
scoresT [kv_j, q] orientation lets the exp tile feed attn@V directly as
the moving operand; a ones-column on V yields softmax denominators in
the same matmul; 1/denominator and bout fold into the projection epilogue.
"""
import numpy as np
from contextlib import ExitStack

import concourse.bass as bass
import concourse.mybir as mybir
import concourse.tile as tile
from concourse import bacc
from concourse.bass_utils import run_bass_kernel_spmd
from concourse.masks import make_identity

F32 = mybir.dt.float32
F32R = mybir.dt.float32r
I32 = mybir.dt.int32
PE = mybir.EngineType.PE
POOL = mybir.EngineType.Pool
DVE = mybir.EngineType.DVE

N, DIM, DH, DOUT = 8192, 1024, 64, 1024
NCORES = 8
QB = 512                 # query block rows
NBLK = N // QB           # 16
JT = 128                 # kv chunk width
NCH = 68                 # chunks per core: 8 diag + 60 off-diag
NSLOT = NCH + 4          # + 4 trash slots for unwanted unpack pieces
UW = 896                 # mask tensor width (diag slices at 384-128t)
SCALE = DH ** -0.5


def R(ap):
    return ap.bitcast(F32R)


def _tables(m: int) -> np.ndarray:
    """Per-core int32 table [1, 192]:
      [0:60)     kc, [60:120) vc for chunks 8..67 (global-layout
                 element offsets into k_full / v_full free dims)
      [120:180)  hq (query-half * QB) for chunks 8..67
      [180:192)  pad
    Chunk order: 0-7 = the two diagonal blocks (local, static);
    8..8+4*b0-1 = off-diag of half 0 (jt ascending);
    then off-diag of half 1.
    """
    b0, b1 = m, NBLK - 1 - m
    kc, vc, hq = [], [], []
    for h, b in ((0, b0), (1, b1)):
        for jt in range(4 * b):
            kc.append(jt * JT)
            vc.append(jt * (DH + 1))
            hq.append(h * QB)
    t = np.array(kc + vc + hq + [0] * 12, dtype=np.int32)
    assert t.shape == (192,)
    return t.reshape(1, 192)


def build():
    nc = bacc.Bacc("TRN2", target_bir_lowering=False, debug=False,
                   num_devices=NCORES)
    x = nc.dram_tensor("x", [2 * QB, DIM], F32, kind="ExternalInput")
    wqkv = nc.dram_tensor("wqkv", [DIM, 3 * DH], F32, kind="ExternalInput")
    wout = nc.dram_tensor("wout", [DH, DOUT], F32, kind="ExternalInput")
    bout = nc.dram_tensor("bout", [DOUT], F32, kind="ExternalInput")
    sched = nc.dram_tensor("sched", [1, 192], I32, kind="ExternalInput")
    out = nc.dram_tensor("out", [2 * QB, DOUT], F32, kind="ExternalOutput")

    with tile.TileContext(nc) as tc, ExitStack() as ctx:
        con = ctx.enter_context(tc.tile_pool(name="con", bufs=1))
        dram = ctx.enter_context(tc.tile_pool(name="dram", bufs=1, space="DRAM"))

        ident = con.tile([128, 128], F32)
        make_identity(nc, ident[:])
        umask_f = con.tile([128, UW], F32)
        nc.gpsimd.memset(umask_f[:], 1.0)
        nc.gpsimd.affine_select(
            out=umask_f[:], in_=umask_f[:], pattern=[[1, UW]],
            channel_multiplier=-1, base=-384,
            compare_op=mybir.AluOpType.is_ge, fill=0.0)
        umask = con.tile([128, UW], F32R)
        nc.vector.tensor_copy(umask[:], umask_f[:])
        ones_f = con.tile([128, 1], F32)
        nc.vector.memset(ones_f[:], 1.0)

        w_f32 = con.tile([128, 8, 3 * DH], F32)
        nc.sync.dma_start(w_f32[:], wqkv.ap().rearrange("(c p) n -> p c n", p=128))
        w_sb = con.tile([128, 8, 3 * DH], F32R)
        nc.vector.tensor_copy(w_sb[:], w_f32[:])
        wo_f32 = con.tile([DH + 1, DOUT], F32)
        nc.sync.dma_start(wo_f32[0:DH, :], wout.ap())
        nc.sync.dma_start(wo_f32[DH:DH + 1, :], bout.ap()[None, :])
        wo_sb = con.tile([DH + 1, DOUT], F32R)
        nc.vector.tensor_copy(wo_sb[:], wo_f32[:])
        t_sb = con.tile([1, 192], I32)
        nc.sync.dma_start(t_sb[:], sched.ap())

        qt_sb = con.tile([DH, 2 * QB], F32R)
        k_full = con.tile([DH, N], F32R)
        v_full = con.tile([128, (N // JT) * (DH + 1)], F32R)
        v_view = v_full.rearrange("p (t c) -> p t c", c=DH + 1)
        nc.vector.tensor_copy(v_view[:, :, DH:DH + 1],
                              ones_f[:, 0:1].to_broadcast((128, N // JT, 1)))
        k_loc = con.tile([DH, 2 * QB], F32R)
        v_loc = con.tile([128, 8 * (DH + 1)], F32R)
        vl_view = v_loc.rearrange("p (t c) -> p t c", c=DH + 1)
        nc.vector.tensor_copy(vl_view[:, :, DH:DH + 1],
                              ones_f[:, 0:1].to_broadcast((128, 8, 1)))

        gin_k = dram.tile([DH, 2 * QB], F32R)
        gout_k = dram.tile([NCORES * DH, 2 * QB], F32R, addr_space="Shared")
        gin_v = dram.tile([128, 8 * DH], F32R)
        gout_v = dram.tile([NCORES * 128, 8 * DH], F32R, addr_space="Shared")

        # ---- phase A: transposed projections of own rows ----
        with tc.tile_pool(name="pa_sb", bufs=1) as pa, \
             tc.tile_pool(name="pa_ps", bufs=3, space="PSUM") as pps, \
             tc.tile_pool(name="pa_ps2", bufs=2, space="PSUM") as pps2:
            x_sb = pa.tile([128, 8, DIM], F32, tag="xin")
            nc.sync.dma_start(x_sb[:], x.ap().rearrange("(t p) d -> p t d", p=128))
            xt_sb = pa.tile([128, 8, 2 * QB], F32R, tag="xt")
            for t in range(8):
                for dc in range(8):
                    tp = pps.tile([128, 128], F32, tag="tp")
                    nc.tensor.transpose(tp[:], x_sb[:, t, dc * 128:(dc + 1) * 128],
                                        ident[:])
                    dst = xt_sb[:, dc, t * 128:(t + 1) * 128]
                    if (t * 8 + dc) % 2 == 0:
                        nc.scalar.copy(dst, tp[:])
                    else:
                        nc.vector.tensor_copy(dst, tp[:])

            # kT_own -> k_loc (feeds diagonal chunks + gather input)
            for h in range(2):
                pq = pps2.tile([DH, QB], F32, tag="pq")
                for dc in range(8):
                    nc.tensor.matmul(
                        pq[:], (w_sb[:, dc, DH:2 * DH]),
                        (xt_sb[:, dc, h * QB:(h + 1) * QB]),
                        start=(dc == 0), stop=(dc == 7))
                nc.scalar.copy(k_loc[:, h * QB:(h + 1) * QB], pq[:])
                nc.sync.dma_start(gin_k[:, h * QB:(h + 1) * QB],
                                  k_loc[:, h * QB:(h + 1) * QB])
            nc.gpsimd.collective_compute(
                "AllGather", mybir.AluOpType.bypass,
                replica_groups=[list(range(NCORES))],
                ins=[gin_k[:]], outs=[gout_k[:]])

            # vT_own then transpose to [rows, dh] slots 0..7 + gather input
            vt_sb = pa.tile([DH, 2 * QB], F32, tag="vt")
            for h in range(2):
                pq = pps2.tile([DH, QB], F32, tag="pq")
                for dc in range(8):
                    nc.tensor.matmul(
                        pq[:], (w_sb[:, dc, 2 * DH:3 * DH]),
                        (xt_sb[:, dc, h * QB:(h + 1) * QB]),
                        start=(dc == 0), stop=(dc == 7))
                nc.vector.tensor_copy(vt_sb[:, h * QB:(h + 1) * QB], pq[:])
            for t in range(8):
                tp = pps.tile([128, 128], F32, tag="tp")
                nc.tensor.transpose(tp[0:128, 0:DH],
                                    vt_sb[:, t * 128:(t + 1) * 128], ident[0:DH, 0:DH])
                nc.scalar.copy(vl_view[:, t, 0:DH], tp[0:128, 0:DH])
                nc.sync.dma_start(gin_v[:, t * DH:(t + 1) * DH], vl_view[:, t, 0:DH])
            nc.gpsimd.collective_compute(
                "AllGather", mybir.AluOpType.bypass,
                replica_groups=[list(range(NCORES))],
                ins=[gin_v[:]], outs=[gout_v[:]])

            # qT
            for h in range(2):
                pq = pps2.tile([DH, QB], F32, tag="pq")
                for dc in range(8):
                    nc.tensor.matmul(
                        pq[:], (w_sb[:, dc, 0:DH]),
                        (xt_sb[:, dc, h * QB:(h + 1) * QB]),
                        start=(dc == 0), stop=(dc == 7))
                nc.scalar.copy(qt_sb[:, h * QB:(h + 1) * QB], pq[:])

            # unpack gathers into GLOBAL column order (static DMAs)
            for g in range(NCORES):
                for sblk_i, blk in ((0, g), (1, NBLK - 1 - g)):
                    src = gout_k[g * DH:(g + 1) * DH,
                                 sblk_i * QB:(sblk_i + 1) * QB]
                    nc.sync.dma_start(
                        k_full[:, blk * QB:(blk + 1) * QB], src)
                    vsrc = gout_v[g * 128:(g + 1) * 128,
                                  sblk_i * 4 * DH:(sblk_i + 1) * 4 * DH]
                    nc.sync.dma_start(
                        v_view[:, 4 * blk:4 * blk + 4, 0:DH],
                        vsrc.rearrange("p (t c) -> p t c", c=DH))

        # ---- phase B: 34 chunk-pairs ----
        with tc.tile_pool(name="pb_st", bufs=2, space="PSUM") as pst, \
             tc.tile_pool(name="pb_oe", bufs=1, space="PSUM") as poe, \
             tc.tile_pool(name="pb_sb", bufs=3) as pb, \
             tc.tile_pool(name="pc_sb", bufs=2) as pc:
            zrow = pc.tile([1, DH + 1], F32, tag="zr")
            zcol = pc.tile([1, QB], F32, tag="zc")
            nc.vector.memset(zrow[:], 0.0)
            nc.vector.memset(zcol[:], 0.0)

            oe = poe.tile([DH + 1, 2 * QB], F32)
            hq_vals = {}
            kc_vals = {}
            vc_vals = {}
            for c in range(NCH):
                if c >= 8 and (c - 8) % 4 == 0:
                    o = c - 8
                    vals = nc.values_load_multi_w_load_instructions(
                        t_sb[0:1, 120 + o:120 + o + 4], engines=[PE],
                        min_val=0, max_val=QB,
                        skip_runtime_bounds_check=True)[1]
                    kv = nc.values_load_multi_w_load_instructions(
                        t_sb[0:1, o:o + 4], engines=[DVE],
                        min_val=0, max_val=N - JT,
                        skip_runtime_bounds_check=True)[1]
                    vv = nc.values_load_multi_w_load_instructions(
                        t_sb[0:1, 60 + o:60 + o + 4], engines=[DVE],
                        min_val=0, max_val=(N // JT - 1) * (DH + 1),
                        skip_runtime_bounds_check=True)[1]
                    for j in range(4):
                        hq_vals[c + j] = vals[j]
                        kc_vals[c + j] = kv[j]
                        vc_vals[c + j] = vv[j]
                if c % 2 == 0:
                    st = pst.tile([128, 2 * QB], F32, tag="st")
                half = c % 2
                stv = st[:, half * QB:(half + 1) * QB]
                if c < 4:
                    qs = qt_sb[:, 0:QB]
                    ks = k_loc[:, c * JT:(c + 1) * JT]
                elif c < 8:
                    qs = qt_sb[:, QB:2 * QB]
                    ks = k_loc[:, QB + (c - 4) * JT:QB + (c - 3) * JT]
                else:
                    qs = qt_sb[:, bass.ds(hq_vals[c], QB)]
                    ks = pb.tile([DH, JT], F32R, tag="kstg")
                    nc.vector.tensor_copy(ks[:], k_full[:, bass.ds(kc_vals[c], JT)])
                    ks = ks[:]
                nc.tensor.matmul(stv, (ks), (qs), start=True, stop=True)
                if c % 2 == 1:
                    et = pb.tile([128, 2 * QB], F32R, tag="et")
                    nc.scalar.activation(et[:], st[:],
                                         mybir.ActivationFunctionType.Exp,
                                         scale=SCALE)
                    for cc in (c - 1, c):
                        hf = cc % 2
                        ev = et[:, hf * QB:(hf + 1) * QB]
                        if cc < 8:
                            mo = 384 - 128 * (cc % 4)
                            nc.vector.tensor_mul(ev, ev, umask[:, mo:mo + QB])
                            vs = vl_view[:, cc, :]
                        else:
                            vstg = pb.tile([128, DH + 1], F32R, tag="vstg")
                            nc.vector.tensor_copy(
                                vstg[:], v_full[:, bass.ds(vc_vals[cc], DH + 1)])
                            vs = vstg[:]
                        if cc < 4:
                            ov = oe[:, 0:QB]
                            strt = cc == 0
                        elif cc < 8:
                            ov = oe[:, QB:2 * QB]
                            strt = cc == 4
                        else:
                            ov = oe[:, bass.ds(hq_vals[cc], QB)]
                            strt = False
                        nc.tensor.matmul(ov, (vs), (ev),
                                         start=strt, stop=False)
            nc.tensor.matmul(oe[:, 0:QB], (zrow[:]), (zcol[:]),
                             start=False, stop=True)
            nc.tensor.matmul(oe[:, QB:2 * QB], (zrow[:]), (zcol[:]),
                             start=False, stop=True)

            # ---- phase C: denominators -> reciprocal in [q,1] layout ----
            oe_sb = pc.tile([DH + 1, 2 * QB], F32, tag="oesb")
            nc.scalar.copy(oe_sb[:], oe[:])
            oe_sbr = pc.tile([DH + 1, 2 * QB], F32R, tag="oesbr")
            nc.vector.tensor_copy(oe_sbr[:], oe[:])
            sums_t = pc.tile([128, 8], F32, tag="sums")
            for s in range(8):
                t1 = pst.tile([128, 1], F32, tag="st")
                nc.tensor.transpose(t1[:], oe_sb[DH:DH + 1, s * 128:(s + 1) * 128],
                                    ident[DH:DH + 1, DH:DH + 1])
                nc.vector.tensor_copy(sums_t[:, s:s + 1], t1[:])
            recip = pc.tile([128, 8], F32, tag="recip")
            nc.vector.reciprocal(recip[:], sums_t[:])

            # ---- phase D: projection + 1/sum scale + bias ----
            for s in range(8):
                pp = poe.tile([128, DOUT], F32, tag="pp")
                for nn in range(2):
                    nc.tensor.matmul(pp[:, nn * QB:(nn + 1) * QB],
                                     oe_sbr[:, s * 128:(s + 1) * 128],
                                     wo_sb[:, nn * QB:(nn + 1) * QB],
                                     start=True, stop=True)
                fo = pb.tile([128, DOUT], F32, tag="fo")
                nc.vector.tensor_scalar_mul(fo[:], pp[:], recip[:, s:s + 1])
                nc.sync.dma_start(out.ap()[s * 128:(s + 1) * 128, :], fo[:])

    nc.compile()
    return nc


_NC_CACHE = None


def kernel(x, Wqkv, Wout, bout):
    global _NC_CACHE
    if _NC_CACHE is None:
        _NC_CACHE = build()
    nc = _NC_CACHE
    x = np.ascontiguousarray(x, dtype=np.float32)
    in_maps = []
    for m in range(NCORES):
        b0, b1 = m, NBLK - 1 - m
        x_own = np.concatenate(
            [x[b0 * QB:(b0 + 1) * QB], x[b1 * QB:(b1 + 1) * QB]], axis=0)
        in_maps.append({
            "x": x_own,
            "wqkv": np.ascontiguousarray(Wqkv, dtype=np.float32),
            "wout": np.ascontiguousarray(Wout, dtype=np.float32),
            "bout": np.ascontiguousarray(bout, dtype=np.float32),
            "sched": _tables(m),
        })
    res = run_bass_kernel_spmd(nc, in_maps, core_ids=list(range(NCORES)))
    out = np.empty((N, DOUT), dtype=np.float32)
    for m in range(NCORES):
        b0, b1 = m, NBLK - 1 - m
        r = res.results[m]["out"]
        out[b0 * QB:(b0 + 1) * QB] = r[0:QB]
        out[b1 * QB:(b1 + 1) * QB] = r[QB:2 * QB]
    return out


def make_in_maps(inputs):
    x = np.ascontiguousarray(inputs["x"], np.float32)
    in_maps = []
    for m in range(NCORES):
        b0, b1 = m, NBLK - 1 - m
        x_own = np.concatenate(
            [x[b0 * QB:(b0 + 1) * QB], x[b1 * QB:(b1 + 1) * QB]], axis=0)
        in_maps.append({
            "x": x_own,
            "wqkv": np.ascontiguousarray(inputs["Wqkv"], np.float32),
            "wout": np.ascontiguousarray(inputs["Wout"], np.float32),
            "bout": np.ascontiguousarray(inputs["bout"], np.float32),
            "sched": _tables(m),
        })
    return in_maps


# revision 14
# speedup vs baseline: 1.8233x; 1.1577x over previous
"""Causal attention kernel for TRN2, 8 NeuronCores, sequence-parallel.

Problem: x[8192,1024] @ Wqkv[1024,192] -> q,k,v[8192,64];
         causal softmax(q k^T / 8) @ v -> [8192,64]; @ Wout[64,1024] + bout.

Sharding: 16 query blocks of 512 rows; core m owns blocks m and 15-m, so
every core processes the same number of unmasked KV columns (17*512).
Each core projects qT/kT/vT for its own 1024 rows (from a host-transposed
xT input, fp32r matmuls), all-gathers kT and v in bf16, then runs a flat
68-chunk scoresT/exp/PV loop (chunk = 128 KV rows x 512 queries) with
bf16 matmul operands and fp32 PSUM accumulation.

The per-core causal structure is folded into DATA so the SPMD instruction
stream is uniform:
  - chunks 0-7 are the two 512x512 diagonal blocks: statically masked,
    fed from locally-computed kT/v, so they run during the gathers;
  - chunks 8-67 are fully-valid off-diagonal chunks (no mask); their
    kT/v column offsets and query-half offset come from an int32 table
    read into registers (batched loads of 4), with tiny DVE staging
    copies making the stationary operands static for the PE.
scoresT [kv_j, q] orientation lets the exp tile feed attn@V directly as
the moving operand; a ones-column on V yields softmax denominators in
the same matmul; 1/denominator and bout fold into the projection epilogue.
"""
import numpy as np
import ml_dtypes
from contextlib import ExitStack

import concourse.bass as bass
import concourse.mybir as mybir
import concourse.tile as tile
from concourse import bacc
from concourse.bass_utils import run_bass_kernel_spmd
from concourse.masks import make_identity

F32 = mybir.dt.float32
F32R = mybir.dt.float32r
BF16 = mybir.dt.bfloat16
I32 = mybir.dt.int32
PE = mybir.EngineType.PE
DVE = mybir.EngineType.DVE

N, DIM, DH, DOUT = 8192, 1024, 64, 1024
NCORES = 8
QB = 512                 # query block rows
NBLK = N // QB           # 16
JT = 128                 # kv chunk width
NCH = 68                 # chunks per core: 8 diag + 60 off-diag
UW = 896                 # mask tensor width (diag slices at 384-128t)
SCALE = DH ** -0.5


def _tables(m: int) -> np.ndarray:
    """Per-core int32 table [1, 192]:
      [0:60)     kc, [60:120) vc for chunks 8..67 (element offsets into
                 k_full / v_full free dims, global column order)
      [120:180)  hq (query-half * QB) for chunks 8..67
      [180:192)  pad
    Chunk order: 0-7 = diagonal blocks of halves 0,1 (local, static);
    then off-diag of half 0 (jt ascending), then off-diag of half 1.
    """
    b0, b1 = m, NBLK - 1 - m
    kc, vc, hq = [], [], []
    for h, b in ((0, b0), (1, b1)):
        for jt in range(4 * b):
            kc.append(jt * JT)
            vc.append(jt * (DH + 1))
            hq.append(h * QB)
    t = np.array(kc + vc + hq + [0] * 12, dtype=np.int32)
    assert t.shape == (192,)
    return t.reshape(1, 192)


def build():
    nc = bacc.Bacc("TRN2", target_bir_lowering=False, debug=False,
                   num_devices=NCORES)
    xt = nc.dram_tensor("xt", [DIM, 2 * QB], F32R, kind="ExternalInput")
    wqkv = nc.dram_tensor("wqkv", [DIM, 3 * DH], F32R, kind="ExternalInput")
    wout = nc.dram_tensor("wout", [DH, DOUT], BF16, kind="ExternalInput")
    bout = nc.dram_tensor("bout", [DOUT], BF16, kind="ExternalInput")
    sched = nc.dram_tensor("sched", [1, 192], I32, kind="ExternalInput")
    out = nc.dram_tensor("out", [2 * QB, DOUT], F32, kind="ExternalOutput")

    with tile.TileContext(nc) as tc, ExitStack() as ctx:
        con = ctx.enter_context(tc.tile_pool(name="con", bufs=1))
        dram = ctx.enter_context(tc.tile_pool(name="dram", bufs=1, space="DRAM"))

        ident = con.tile([128, 128], F32)
        make_identity(nc, ident[:])
        umask_f = con.tile([128, UW], F32)
        nc.gpsimd.memset(umask_f[:], 1.0)
        nc.gpsimd.affine_select(
            out=umask_f[:], in_=umask_f[:], pattern=[[1, UW]],
            channel_multiplier=-1, base=-384,
            compare_op=mybir.AluOpType.is_ge, fill=0.0)
        umask = con.tile([128, UW], BF16)
        nc.vector.tensor_copy(umask[:], umask_f[:])
        ones_f = con.tile([128, 1], F32)
        nc.vector.memset(ones_f[:], 1.0)

        w_sb = con.tile([128, 8, 3 * DH], F32R)
        nc.sync.dma_start(w_sb[:], wqkv.ap().rearrange("(c p) n -> p c n", p=128))
        wo_sb = con.tile([DH + 1, DOUT], BF16)
        nc.sync.dma_start(wo_sb[0:DH, :], wout.ap())
        nc.sync.dma_start(wo_sb[DH:DH + 1, :], bout.ap()[None, :])
        t_sb = con.tile([1, 192], I32)
        nc.sync.dma_start(t_sb[:], sched.ap())

        qt_sb = con.tile([DH, 2 * QB], BF16)
        k_full = con.tile([DH, N], BF16)
        v_full = con.tile([128, (N // JT) * (DH + 1)], BF16)
        v_view = v_full.rearrange("p (t c) -> p t c", c=DH + 1)
        nc.vector.tensor_copy(v_view[:, :, DH:DH + 1],
                              ones_f[:, 0:1].to_broadcast((128, N // JT, 1)))
        k_loc = con.tile([DH, 2 * QB], BF16)
        v_loc = con.tile([128, 8 * (DH + 1)], BF16)
        vl_view = v_loc.rearrange("p (t c) -> p t c", c=DH + 1)
        nc.vector.tensor_copy(vl_view[:, :, DH:DH + 1],
                              ones_f[:, 0:1].to_broadcast((128, 8, 1)))

        gin_k = dram.tile([DH, 2 * QB], BF16)
        gout_k = dram.tile([NCORES * DH, 2 * QB], BF16, addr_space="Shared")
        gin_v = dram.tile([128, 8 * DH], BF16)
        gout_v = dram.tile([NCORES * 128, 8 * DH], BF16, addr_space="Shared")

        # ---- phase A: projections of own rows from host-transposed xT ----
        with tc.tile_pool(name="pa_sb", bufs=1) as pa, \
             tc.tile_pool(name="pa_ps", bufs=2, space="PSUM") as pps, \
             tc.tile_pool(name="pa_ps2", bufs=2, space="PSUM") as pps2:
            xt_sb = pa.tile([128, 8, 2 * QB], F32R, tag="xt")
            nc.sync.dma_start(xt_sb[:], xt.ap().rearrange("(c p) r -> p c r", p=128))

            # kT_own -> k_loc (bf16; feeds diagonal chunks + gather input)
            for h in range(2):
                pq = pps2.tile([DH, QB], F32, tag="pq")
                for dc in range(8):
                    nc.tensor.matmul(
                        pq[:], w_sb[:, dc, DH:2 * DH],
                        xt_sb[:, dc, h * QB:(h + 1) * QB],
                        start=(dc == 0), stop=(dc == 7))
                nc.scalar.copy(k_loc[:, h * QB:(h + 1) * QB], pq[:])
                nc.sync.dma_start(gin_k[:, h * QB:(h + 1) * QB],
                                  k_loc[:, h * QB:(h + 1) * QB])
            nc.gpsimd.collective_compute(
                "AllGather", mybir.AluOpType.bypass,
                replica_groups=[list(range(NCORES))],
                ins=[gin_k[:]], outs=[gout_k[:]])

            # vT_own, then transpose to [rows, dh] -> v_loc + gather input
            vt_sb = pa.tile([DH, 2 * QB], F32, tag="vt")
            for h in range(2):
                pq = pps2.tile([DH, QB], F32, tag="pq")
                for dc in range(8):
                    nc.tensor.matmul(
                        pq[:], w_sb[:, dc, 2 * DH:3 * DH],
                        xt_sb[:, dc, h * QB:(h + 1) * QB],
                        start=(dc == 0), stop=(dc == 7))
                nc.vector.tensor_copy(vt_sb[:, h * QB:(h + 1) * QB], pq[:])
            for t in range(8):
                tp = pps.tile([128, 128], F32, tag="tp")
                nc.tensor.transpose(tp[0:128, 0:DH],
                                    vt_sb[:, t * 128:(t + 1) * 128],
                                    ident[0:DH, 0:DH])
                nc.scalar.copy(vl_view[:, t, 0:DH], tp[0:128, 0:DH])
                nc.sync.dma_start(gin_v[:, t * DH:(t + 1) * DH],
                                  vl_view[:, t, 0:DH])
            nc.gpsimd.collective_compute(
                "AllGather", mybir.AluOpType.bypass,
                replica_groups=[list(range(NCORES))],
                ins=[gin_v[:]], outs=[gout_v[:]])

            # qT (bf16)
            for h in range(2):
                pq = pps2.tile([DH, QB], F32, tag="pq")
                for dc in range(8):
                    nc.tensor.matmul(
                        pq[:], w_sb[:, dc, 0:DH],
                        xt_sb[:, dc, h * QB:(h + 1) * QB],
                        start=(dc == 0), stop=(dc == 7))
                nc.scalar.copy(qt_sb[:, h * QB:(h + 1) * QB], pq[:])

        # unpack gathers into GLOBAL column order (static DMAs, con scope)
        for g in range(NCORES):
            for sblk_i, blk in ((0, g), (1, NBLK - 1 - g)):
                src = gout_k[g * DH:(g + 1) * DH,
                             sblk_i * QB:(sblk_i + 1) * QB]
                nc.sync.dma_start(k_full[:, blk * QB:(blk + 1) * QB], src)
                vsrc = gout_v[g * 128:(g + 1) * 128,
                              sblk_i * 4 * DH:(sblk_i + 1) * 4 * DH]
                nc.sync.dma_start(
                    v_view[:, 4 * blk:4 * blk + 4, 0:DH],
                    vsrc.rearrange("p (t c) -> p t c", c=DH))

        # ---- phase B: 34 chunk-pairs of scoresT -> exp -> [mask] -> PV ----
        with tc.tile_pool(name="pb_st", bufs=2, space="PSUM") as pst, \
             tc.tile_pool(name="pb_oe", bufs=1, space="PSUM") as poe, \
             tc.tile_pool(name="pb_pp", bufs=2, space="PSUM") as ppp, \
             tc.tile_pool(name="pb_sb", bufs=3) as pb, \
             tc.tile_pool(name="pc_sb", bufs=2) as pc:
            zrow = pc.tile([1, DH + 1], F32, tag="zr")
            zcol = pc.tile([1, QB], F32, tag="zc")
            nc.vector.memset(zrow[:], 0.0)
            nc.vector.memset(zcol[:], 0.0)

            oe = poe.tile([DH + 1, 2 * QB], F32)
            hq_vals, kc_vals, vc_vals = {}, {}, {}
            st = None
            for c in range(NCH):
                if c >= 8 and (c - 8) % 4 == 0:
                    o = c - 8
                    vals = nc.values_load_multi_w_load_instructions(
                        t_sb[0:1, 120 + o:120 + o + 4], engines=[PE],
                        min_val=0, max_val=QB,
                        skip_runtime_bounds_check=True)[1]
                    kv = nc.values_load_multi_w_load_instructions(
                        t_sb[0:1, o:o + 4], engines=[DVE],
                        min_val=0, max_val=N - JT,
                        skip_runtime_bounds_check=True)[1]
                    vv = nc.values_load_multi_w_load_instructions(
                        t_sb[0:1, 60 + o:60 + o + 4], engines=[DVE],
                        min_val=0, max_val=(N // JT - 1) * (DH + 1),
                        skip_runtime_bounds_check=True)[1]
                    for j in range(4):
                        hq_vals[c + j] = vals[j]
                        kc_vals[c + j] = kv[j]
                        vc_vals[c + j] = vv[j]
                if c % 2 == 0:
                    st = pst.tile([128, 2 * QB], F32, tag="st")
                half = c % 2
                stv = st[:, half * QB:(half + 1) * QB]
                if c < 4:
                    qs = qt_sb[:, 0:QB]
                    ks = k_loc[:, c * JT:(c + 1) * JT]
                elif c < 8:
                    qs = qt_sb[:, QB:2 * QB]
                    ks = k_loc[:, QB + (c - 4) * JT:QB + (c - 3) * JT]
                else:
                    qs = qt_sb[:, bass.ds(hq_vals[c], QB)]
                    kst = pb.tile([DH, JT], BF16, tag="kstg")
                    nc.vector.tensor_copy(kst[:],
                                          k_full[:, bass.ds(kc_vals[c], JT)])
                    ks = kst[:]
                nc.tensor.matmul(stv, ks, qs, start=True, stop=True)
                if c % 2 == 1:
                    et = pb.tile([128, 2 * QB], BF16, tag="et")
                    nc.scalar.activation(et[:], st[:],
                                         mybir.ActivationFunctionType.Exp,
                                         scale=SCALE)
                    for cc in (c - 1, c):
                        hf = cc % 2
                        ev = et[:, hf * QB:(hf + 1) * QB]
                        if cc < 8:
                            mo = 384 - 128 * (cc % 4)
                            nc.vector.tensor_mul(ev, ev, umask[:, mo:mo + QB])
                            vs = vl_view[:, cc, :]
                        else:
                            vstg = pb.tile([128, DH + 1], BF16, tag="vstg")
                            nc.vector.tensor_copy(
                                vstg[:], v_full[:, bass.ds(vc_vals[cc], DH + 1)])
                            vs = vstg[:]
                        if cc < 4:
                            ov = oe[:, 0:QB]
                            strt = cc == 0
                        elif cc < 8:
                            ov = oe[:, QB:2 * QB]
                            strt = cc == 4
                        else:
                            ov = oe[:, bass.ds(hq_vals[cc], QB)]
                            strt = False
                        nc.tensor.matmul(ov, vs, ev, start=strt, stop=False)
            nc.tensor.matmul(oe[:, 0:QB], zrow[:], zcol[:],
                             start=False, stop=True)
            nc.tensor.matmul(oe[:, QB:2 * QB], zrow[:], zcol[:],
                             start=False, stop=True)

            # ---- phase C: denominators -> reciprocal in [q,1] layout ----
            oe_sb = pc.tile([DH + 1, 2 * QB], F32, tag="oesb")
            nc.scalar.copy(oe_sb[:], oe[:])
            oe_sbr = pc.tile([DH + 1, 2 * QB], BF16, tag="oesbr")
            nc.vector.tensor_copy(oe_sbr[:], oe[:])
            sums_t = pc.tile([128, 8], F32, tag="sums")
            for s in range(8):
                t1 = pst.tile([128, 1], F32, tag="st")
                nc.tensor.transpose(t1[:], oe_sb[DH:DH + 1, s * 128:(s + 1) * 128],
                                    ident[DH:DH + 1, DH:DH + 1])
                nc.vector.tensor_copy(sums_t[:, s:s + 1], t1[:])
            recip = pc.tile([128, 8], F32, tag="recip")
            nc.vector.reciprocal(recip[:], sums_t[:])

            # ---- phase D: projection + 1/sum scale + bias ----
            for s in range(8):
                for nn in range(2):
                    pp = ppp.tile([128, QB], F32, tag="pp")
                    nc.tensor.matmul(pp[:],
                                     oe_sbr[:, s * 128:(s + 1) * 128],
                                     wo_sb[:, nn * QB:(nn + 1) * QB],
                                     start=True, stop=True)
                    fo = pb.tile([128, QB], F32, tag="fo")
                    nc.vector.tensor_scalar_mul(fo[:], pp[:], recip[:, s:s + 1])
                    nc.sync.dma_start(
                        out.ap()[s * 128:(s + 1) * 128, nn * QB:(nn + 1) * QB],
                        fo[:])

    nc.compile()
    return nc


_NC_CACHE = None


def make_in_maps(inputs):
    x = np.ascontiguousarray(inputs["x"], np.float32)
    wqkv = np.ascontiguousarray(inputs["Wqkv"], np.float32)
    wout = np.asarray(inputs["Wout"]).astype(ml_dtypes.bfloat16)
    bout_b = np.asarray(inputs["bout"]).astype(ml_dtypes.bfloat16)
    in_maps = []
    for m in range(NCORES):
        b0, b1 = m, NBLK - 1 - m
        x_own = np.concatenate(
            [x[b0 * QB:(b0 + 1) * QB], x[b1 * QB:(b1 + 1) * QB]], axis=0)
        in_maps.append({
            "xt": np.ascontiguousarray(x_own.T),
            "wqkv": wqkv,
            "wout": wout,
            "bout": bout_b,
            "sched": _tables(m),
        })
    return in_maps


def kernel(x, Wqkv, Wout, bout):
    global _NC_CACHE
    if _NC_CACHE is None:
        _NC_CACHE = build()
    nc = _NC_CACHE
    in_maps = make_in_maps({"x": x, "Wqkv": Wqkv, "Wout": Wout, "bout": bout})
    res = run_bass_kernel_spmd(nc, in_maps, core_ids=list(range(NCORES)))
    out = np.empty((N, DOUT), dtype=np.float32)
    for m in range(NCORES):
        b0, b1 = m, NBLK - 1 - m
        r = res.results[m]["out"]
        out[b0 * QB:(b0 + 1) * QB] = r[0:QB]
        out[b1 * QB:(b1 + 1) * QB] = r[QB:2 * QB]
    return out


# revision 17
# speedup vs baseline: 1.9920x; 1.0925x over previous
"""Causal attention kernel for TRN2, 8 NeuronCores, sequence-parallel.

Problem: x[8192,1024] @ Wqkv[1024,192] -> q,k,v[8192,64];
         causal softmax(q k^T / 8) @ v -> [8192,64]; @ Wout[64,1024] + bout.

Sharding: 16 query blocks of 512 rows; core m owns blocks m and 15-m, so
every core processes the same number of unmasked KV columns (17*512).
Each core projects qT/kT/vT for its own 1024 rows (from a host-transposed
xT input, fp32r matmuls), all-gathers kT and v in bf16, then runs a flat
68-chunk scoresT/exp/PV loop (chunk = 128 KV rows x 512 queries) with
bf16 matmul operands and fp32 PSUM accumulation.

The per-core causal structure is folded into DATA so the SPMD instruction
stream is uniform:
  - chunks 0-7 are the two 512x512 diagonal blocks: statically masked,
    fed from locally-computed kT/v, so they run during the gathers;
  - chunks 8-67 are fully-valid off-diagonal chunks (no mask); their
    kT/v column offsets and query-half offset come from an int32 table
    read into registers (batched loads of 4), with tiny DVE staging
    copies making the stationary operands static for the PE.
scoresT [kv_j, q] orientation lets the exp tile feed attn@V directly as
the moving operand; a ones-column on V yields softmax denominators in
the same matmul; 1/denominator and bout fold into the projection epilogue.
"""
import numpy as np
import ml_dtypes
from contextlib import ExitStack

import concourse.bass as bass
import concourse.mybir as mybir
import concourse.tile as tile
from concourse import bacc
from concourse.bass_utils import run_bass_kernel_spmd
from concourse.masks import make_identity

F32 = mybir.dt.float32
F32R = mybir.dt.float32r
BF16 = mybir.dt.bfloat16
I32 = mybir.dt.int32
PE = mybir.EngineType.PE
DVE = mybir.EngineType.DVE

N, DIM, DH, DOUT = 8192, 1024, 64, 1024
NCORES = 8
QB = 512                 # query block rows
NBLK = N // QB           # 16
JT = 128                 # kv chunk width
NCH = 68                 # chunks per core: 8 diag + 60 off-diag
UW = 896                 # mask tensor width (diag slices at 384-128t)
NWARM = 96               # PE warm-up dummies bridging the gather stall
SCALE = DH ** -0.5


def _tables(m: int) -> np.ndarray:
    """Per-core int32 table [1, 192]:
      [0:56)     kc, [56:112) vc for chunks 12..67 (element offsets into
                 k_full / v_full free dims, global column order)
      [112:168)  hq (query-half * QB) for chunks 12..67
      [168:192)  pad
    Chunk order: 0-7 = diagonal blocks of halves 0,1; 8-11 = block b0
    as seen by half 1 (all local/static); then off-diag of half 0
    (jt ascending), then off-diag of half 1 (minus block b0).
    """
    b0, b1 = m, NBLK - 1 - m
    kc, vc, hq = [], [], []
    for h, b in ((0, b0), (1, b1)):
        for jt in range(4 * b):
            if h == 1 and 4 * b0 <= jt < 4 * b0 + 4:
                continue  # chunks 8-11: block b0 for half 1, served locally
            kc.append(jt * JT)
            vc.append(jt * (DH + 1))
            hq.append(h * QB)
    assert len(kc) == 56
    t = np.array(kc + vc + hq + [0] * 24, dtype=np.int32)
    assert t.shape == (192,)
    return t.reshape(1, 192)


def build():
    nc = bacc.Bacc("TRN2", target_bir_lowering=False, debug=False,
                   num_devices=NCORES)
    xt = nc.dram_tensor("xt", [DIM, 2 * QB], BF16, kind="ExternalInput")
    wqkv = nc.dram_tensor("wqkv", [DIM, 3 * DH], BF16, kind="ExternalInput")
    wout = nc.dram_tensor("wout", [DH, DOUT], BF16, kind="ExternalInput")
    bout = nc.dram_tensor("bout", [DOUT], BF16, kind="ExternalInput")
    sched = nc.dram_tensor("sched", [1, 192], I32, kind="ExternalInput")
    out = nc.dram_tensor("out", [2 * QB, DOUT], F32, kind="ExternalOutput")

    with tile.TileContext(nc) as tc, ExitStack() as ctx:
        con = ctx.enter_context(tc.tile_pool(name="con", bufs=1))
        dram = ctx.enter_context(tc.tile_pool(name="dram", bufs=1, space="DRAM"))

        ident = con.tile([128, 128], F32)
        make_identity(nc, ident[:])
        umask_f = con.tile([128, UW], F32)
        nc.gpsimd.memset(umask_f[:], 1.0)
        nc.gpsimd.affine_select(
            out=umask_f[:], in_=umask_f[:], pattern=[[1, UW]],
            channel_multiplier=-1, base=-384,
            compare_op=mybir.AluOpType.is_ge, fill=0.0)
        umask = con.tile([128, UW], BF16)
        nc.vector.tensor_copy(umask[:], umask_f[:])
        ones_f = con.tile([128, 1], F32)
        nc.vector.memset(ones_f[:], 1.0)

        w_sb = con.tile([128, 8, 3 * DH], BF16)
        nc.sync.dma_start(w_sb[:], wqkv.ap().rearrange("(c p) n -> p c n", p=128))
        wo_sb = con.tile([DH + 1, DOUT], BF16)
        nc.sync.dma_start(wo_sb[0:DH, :], wout.ap())
        nc.sync.dma_start(wo_sb[DH:DH + 1, :], bout.ap()[None, :])
        t_sb = con.tile([1, 192], I32)
        nc.sync.dma_start(t_sb[:], sched.ap())

        qt_sb = con.tile([DH, 2 * QB], BF16)
        k_full = con.tile([DH, N], BF16)
        v_full = con.tile([128, (N // JT) * (DH + 1)], BF16)
        v_view = v_full.rearrange("p (t c) -> p t c", c=DH + 1)
        nc.vector.tensor_copy(v_view[:, :, DH:DH + 1],
                              ones_f[:, 0:1].to_broadcast((128, N // JT, 1)))
        k_loc = con.tile([DH, 2 * QB], BF16)
        v_loc = con.tile([128, 8 * (DH + 1)], BF16)
        vl_view = v_loc.rearrange("p (t c) -> p t c", c=DH + 1)
        nc.vector.tensor_copy(vl_view[:, :, DH:DH + 1],
                              ones_f[:, 0:1].to_broadcast((128, 8, 1)))

        gin_k = dram.tile([DH, 2 * QB], BF16)
        gout_k = dram.tile([NCORES * DH, 2 * QB], BF16, addr_space="Shared")
        gin_v = dram.tile([128, 8 * DH], BF16)
        gout_v = dram.tile([NCORES * 128, 8 * DH], BF16, addr_space="Shared")

        # ---- phase A: projections of own rows from host-transposed xT ----
        with tc.tile_pool(name="pa_sb", bufs=1) as pa, \
             tc.tile_pool(name="pa_ps", bufs=2, space="PSUM") as pps, \
             tc.tile_pool(name="pa_ps2", bufs=2, space="PSUM") as pps2:
            xt_sb = pa.tile([128, 8, 2 * QB], BF16, tag="xt")
            xt_r = xt.ap().rearrange("(c p) r -> p c r", p=128)
            for dc in range(8):
                nc.sync.dma_start(xt_sb[:, dc, :], xt_r[:, dc, :])

            # kT_own -> k_loc (bf16; feeds diagonal chunks + gather input)
            for h in range(2):
                pq = pps2.tile([DH, QB], F32, tag="pq")
                for dc in range(8):
                    nc.tensor.matmul(
                        pq[:], w_sb[:, dc, DH:2 * DH],
                        xt_sb[:, dc, h * QB:(h + 1) * QB],
                        start=(dc == 0), stop=(dc == 7))
                nc.scalar.copy(k_loc[:, h * QB:(h + 1) * QB], pq[:])
                nc.sync.dma_start(gin_k[:, h * QB:(h + 1) * QB],
                                  k_loc[:, h * QB:(h + 1) * QB])
            nc.gpsimd.collective_compute(
                "AllGather", mybir.AluOpType.bypass,
                replica_groups=[list(range(NCORES))],
                ins=[gin_k[:]], outs=[gout_k[:]])

            # vT_own, then transpose to [rows, dh] -> v_loc + gather input
            vt_sb = pa.tile([DH, 2 * QB], F32, tag="vt")
            for h in range(2):
                pq = pps2.tile([DH, QB], F32, tag="pq")
                for dc in range(8):
                    nc.tensor.matmul(
                        pq[:], w_sb[:, dc, 2 * DH:3 * DH],
                        xt_sb[:, dc, h * QB:(h + 1) * QB],
                        start=(dc == 0), stop=(dc == 7))
                nc.vector.tensor_copy(vt_sb[:, h * QB:(h + 1) * QB], pq[:])
            for t in range(8):
                tp = pps.tile([128, 128], F32, tag="tp")
                nc.tensor.transpose(tp[0:128, 0:DH],
                                    vt_sb[:, t * 128:(t + 1) * 128],
                                    ident[0:DH, 0:DH])
                nc.scalar.copy(vl_view[:, t, 0:DH], tp[0:128, 0:DH])
                nc.sync.dma_start(gin_v[:, t * DH:(t + 1) * DH],
                                  vl_view[:, t, 0:DH])
            nc.gpsimd.collective_compute(
                "AllGather", mybir.AluOpType.bypass,
                replica_groups=[list(range(NCORES))],
                ins=[gin_v[:]], outs=[gout_v[:]])

            # qT (bf16)
            for h in range(2):
                pq = pps2.tile([DH, QB], F32, tag="pq")
                for dc in range(8):
                    nc.tensor.matmul(
                        pq[:], w_sb[:, dc, 0:DH],
                        xt_sb[:, dc, h * QB:(h + 1) * QB],
                        start=(dc == 0), stop=(dc == 7))
                nc.scalar.copy(qt_sb[:, h * QB:(h + 1) * QB], pq[:])

        # unpack gathers into GLOBAL column order (static DMAs, con scope)
        for g in range(NCORES):
            for sblk_i, blk in ((0, g), (1, NBLK - 1 - g)):
                src = gout_k[g * DH:(g + 1) * DH,
                             sblk_i * QB:(sblk_i + 1) * QB]
                nc.sync.dma_start(k_full[:, blk * QB:(blk + 1) * QB], src)
                vsrc = gout_v[g * 128:(g + 1) * 128,
                              sblk_i * 4 * DH:(sblk_i + 1) * 4 * DH]
                nc.sync.dma_start(
                    v_view[:, 4 * blk:4 * blk + 4, 0:DH],
                    vsrc.rearrange("p (t c) -> p t c", c=DH))

        # ---- phase B: 34 chunk-pairs of scoresT -> exp -> [mask] -> PV ----
        with tc.tile_pool(name="pb_st", bufs=2, space="PSUM") as pst, \
             tc.tile_pool(name="pb_oe", bufs=1, space="PSUM") as poe, \
             tc.tile_pool(name="pb_pp", bufs=2, space="PSUM") as ppp, \
             tc.tile_pool(name="pb_sb", bufs=3) as pb, \
             tc.tile_pool(name="pc_sb", bufs=2) as pc:
            zrow = pc.tile([1, DH + 1], F32, tag="zr")
            zcol = pc.tile([1, QB], F32, tag="zc")
            nc.vector.memset(zrow[:], 0.0)
            nc.vector.memset(zcol[:], 0.0)

            oe = poe.tile([DH + 1, 2 * QB], F32)
            dm_l = pc.tile([1, 1], BF16, tag="dml")
            dm_r = pc.tile([1, QB], BF16, tag="dmr")
            nc.vector.memset(dm_l[:], 0.0)
            nc.vector.memset(dm_r[:], 0.0)
            dmo = None
            hq_vals, kc_vals, vc_vals = {}, {}, {}
            st = None
            for c in range(NCH):
                if c == 12:
                    dmo = ppp.tile([1, QB], F32, tag="pp")
                    for _ in range(NWARM):
                        nc.tensor.matmul(dmo[:], dm_l[:], dm_r[:],
                                         start=True, stop=True)
                if c >= 12 and (c - 12) % 4 == 0:
                    o = c - 12
                    vals = nc.values_load_multi_w_load_instructions(
                        t_sb[0:1, 112 + o:112 + o + 4], engines=[PE],
                        min_val=0, max_val=QB,
                        skip_runtime_bounds_check=True)[1]
                    kv = nc.values_load_multi_w_load_instructions(
                        t_sb[0:1, o:o + 4], engines=[DVE],
                        min_val=0, max_val=N - JT,
                        skip_runtime_bounds_check=True)[1]
                    vv = nc.values_load_multi_w_load_instructions(
                        t_sb[0:1, 56 + o:56 + o + 4], engines=[DVE],
                        min_val=0, max_val=(N // JT - 1) * (DH + 1),
                        skip_runtime_bounds_check=True)[1]
                    for j in range(4):
                        hq_vals[c + j] = vals[j]
                        kc_vals[c + j] = kv[j]
                        vc_vals[c + j] = vv[j]
                if c % 2 == 0:
                    st = pst.tile([128, 2 * QB], F32, tag="st")
                half = c % 2
                stv = st[:, half * QB:(half + 1) * QB]
                if c < 4:
                    qs = qt_sb[:, 0:QB]
                    ks = k_loc[:, c * JT:(c + 1) * JT]
                elif c < 8:
                    qs = qt_sb[:, QB:2 * QB]
                    ks = k_loc[:, QB + (c - 4) * JT:QB + (c - 3) * JT]
                elif c < 12:
                    qs = qt_sb[:, QB:2 * QB]
                    ks = k_loc[:, (c - 8) * JT:(c - 7) * JT]
                else:
                    qs = qt_sb[:, bass.ds(hq_vals[c], QB)]
                    kst = pb.tile([DH, JT], BF16, tag="kstg")
                    nc.vector.tensor_copy(kst[:],
                                          k_full[:, bass.ds(kc_vals[c], JT)])
                    ks = kst[:]
                nc.tensor.matmul(stv, ks, qs, start=True, stop=True)
                if c % 2 == 1:
                    et = pb.tile([128, 2 * QB], BF16, tag="et")
                    nc.scalar.activation(et[:], st[:],
                                         mybir.ActivationFunctionType.Exp,
                                         scale=SCALE)
                    for cc in (c - 1, c):
                        hf = cc % 2
                        ev = et[:, hf * QB:(hf + 1) * QB]
                        if cc < 8:
                            mo = 384 - 128 * (cc % 4)
                            nc.vector.tensor_mul(ev, ev, umask[:, mo:mo + QB])
                            vs = vl_view[:, cc, :]
                        elif cc < 12:
                            vs = vl_view[:, cc - 8, :]
                        else:
                            vstg = pb.tile([128, DH + 1], BF16, tag="vstg")
                            nc.vector.tensor_copy(
                                vstg[:], v_full[:, bass.ds(vc_vals[cc], DH + 1)])
                            vs = vstg[:]
                        if cc < 4:
                            ov = oe[:, 0:QB]
                            strt = cc == 0
                        elif cc < 12:
                            ov = oe[:, QB:2 * QB]
                            strt = cc == 4
                        else:
                            ov = oe[:, bass.ds(hq_vals[cc], QB)]
                            strt = False
                        nc.tensor.matmul(ov, vs, ev, start=strt, stop=False)
            nc.tensor.matmul(oe[:, 0:QB], zrow[:], zcol[:],
                             start=False, stop=True)
            nc.tensor.matmul(oe[:, QB:2 * QB], zrow[:], zcol[:],
                             start=False, stop=True)

            # ---- phase C: denominators -> reciprocal in [q,1] layout ----
            oe_sb = pc.tile([DH + 1, 2 * QB], F32, tag="oesb")
            nc.scalar.copy(oe_sb[:], oe[:])
            oe_sbr = pc.tile([DH + 1, 2 * QB], BF16, tag="oesbr")
            nc.vector.tensor_copy(oe_sbr[:], oe[:])
            sums_t = pc.tile([128, 8], F32, tag="sums")
            for s in range(8):
                t1 = pst.tile([128, 1], F32, tag="st")
                nc.tensor.transpose(t1[:], oe_sb[DH:DH + 1, s * 128:(s + 1) * 128],
                                    ident[DH:DH + 1, DH:DH + 1])
                nc.vector.tensor_copy(sums_t[:, s:s + 1], t1[:])
            recip = pc.tile([128, 8], F32, tag="recip")
            nc.vector.reciprocal(recip[:], sums_t[:])

            # ---- phase D: projection + 1/sum scale + bias ----
            for s in range(8):
                for nn in range(2):
                    pp = ppp.tile([128, QB], F32, tag="pp")
                    nc.tensor.matmul(pp[:],
                                     oe_sbr[:, s * 128:(s + 1) * 128],
                                     wo_sb[:, nn * QB:(nn + 1) * QB],
                                     start=True, stop=True)
                    fo = pb.tile([128, QB], F32, tag="fo")
                    if (2 * s + nn) % 2 == 0:
                        nc.vector.tensor_scalar_mul(fo[:], pp[:],
                                                    recip[:, s:s + 1])
                    else:
                        nc.scalar.activation(fo[:], pp[:],
                                             mybir.ActivationFunctionType.Copy,
                                             scale=recip[:, s:s + 1])
                    nc.sync.dma_start(
                        out.ap()[s * 128:(s + 1) * 128, nn * QB:(nn + 1) * QB],
                        fo[:])

    nc.compile()
    return nc


_NC_CACHE = None


def make_in_maps(inputs):
    x = np.ascontiguousarray(inputs["x"], np.float32)
    wqkv = np.ascontiguousarray(
        np.asarray(inputs["Wqkv"]).astype(ml_dtypes.bfloat16))
    wout = np.asarray(inputs["Wout"]).astype(ml_dtypes.bfloat16)
    bout_b = np.asarray(inputs["bout"]).astype(ml_dtypes.bfloat16)
    in_maps = []
    for m in range(NCORES):
        b0, b1 = m, NBLK - 1 - m
        x_own = np.concatenate(
            [x[b0 * QB:(b0 + 1) * QB], x[b1 * QB:(b1 + 1) * QB]], axis=0)
        in_maps.append({
            "xt": np.ascontiguousarray(x_own.T.astype(ml_dtypes.bfloat16)),
            "wqkv": wqkv,
            "wout": wout,
            "bout": bout_b,
            "sched": _tables(m),
        })
    return in_maps


def kernel(x, Wqkv, Wout, bout):
    global _NC_CACHE
    if _NC_CACHE is None:
        _NC_CACHE = build()
    nc = _NC_CACHE
    in_maps = make_in_maps({"x": x, "Wqkv": Wqkv, "Wout": Wout, "bout": bout})
    res = run_bass_kernel_spmd(nc, in_maps, core_ids=list(range(NCORES)))
    out = np.empty((N, DOUT), dtype=np.float32)
    for m in range(NCORES):
        b0, b1 = m, NBLK - 1 - m
        r = res.results[m]["out"]
        out[b0 * QB:(b0 + 1) * QB] = r[0:QB]
        out[b1 * QB:(b1 + 1) * QB] = r[QB:2 * QB]
    return out
